# revision 3
# baseline (speedup 1.0000x reference)
"""Batched Bjorck orthogonalization on 8 TRN2 NeuronCores (fp8 DoubleRow).

w: [64, 1024, 1024] f32. 13 iterations of W <- 1.5 W - 0.5 W (W^T W).
Batch sharded across 8 cores (8 matrices/core), PAIR=2 software-pipelined.

Schedule (MODES): iteration 0 in fp16 (early iterations sit in the
|f'(sigma)|>1 amplification region near sigma~2 and any noise there is
amplified ~3x/iteration); iterations 1-2 with fp8 G + fp16 U (G noise is
a symmetric perturbation, healed quadratically by later iterations);
iterations 3-11 fully fp8; final iteration with fp16 G + fp8 U (the last
G's noise would reach the output unhealed, while U's error scales with
||I-G|| ~ 1e-6 there).

fp8 iterations use the correction form W' = W + 0.5 W (I-G) with state
V = 128 W in fp16, so fp8 noise enters either proportional to ||I-G||
(decaying) or symmetrically (self-healing). Exact power-of-2 scales
(W8 = q8(V/16), M8 = q8(8(I-G)), drain -1/8 of the 64G PSUM) make the
U-phase PSUM equal the correction exactly: the drain is a single DVE
tensor_tensor add per 512-column chunk.

Matmuls run in fp8 e4m3 DoubleRow perf mode (0.5 PE cycles/row, 256-deep
contraction = 4x fp16 throughput) over chunk-pair operand slices
[p, 2, n]. A data-carrying start=True DoubleRow matmul loses its
contribution on this hardware, so every PSUM accumulation group is
zeroed first by a dummy zero-weights DoubleRow matmul. G8 computes only
the upper triangle (4608/8192 columns); strict-lower 128-blocks are PE
transposes of their mirrors (fp8 transpose lands in u16 lanes; strided
PSUM drain, one batched drain per row-chunk). WT8 (the U stationary)
comes from the XBAR DMA transpose of V (fp16) + quantize, pipelined
per chunk inside the U8 drain loop so the next iteration's operands are
ready with minimal tail latency. Engine placement spreads drains and
quantizes across Act/DVE/Pool (GPSIMD cannot touch PSUM; it takes the
SBUF-only casts and diagonal adds).

Measured on hardware + TimelineSim of the exact program (all 8 cores
run the same program): 2.523 ms, rel err 1.19e-2 vs the fp32 reference
(gate 2e-2). Baseline fp16 kernel: 4.471 ms (1.77x).
"""

import numpy as np

_NC_CACHE = {}

P = 128
NMAT = 1024
C = 8
FB = 512
ITERS = 13

# scaling: V = CS*w (fp16 state); W8 = q8(AS*V); M8 = q8(SM*(I-G)).
# AS*SM = 0.5 makes the U-phase PSUM the exact correction (V += PSUM).
CS = 128.0
SM = 8.0  # '8' / 'g16u8' iterations
AS = 0.5 / SM
SM2 = 2.0  # 'hilo' iterations (2-product hi/lo fp8)
AS2 = 0.5 / SM2


def _asq(mode):
    return AS2 if mode == "hilo" else AS

# per-iteration mode schedule: '16' = fp16 iter (baseline style),
# '8' = fp8 G+U, 'g16u8' = fp16 G + fp8 U (for the final iteration)
MODES = ["16"] + ["g8u16"] * 2 + ["8"] * 9 + ["g16u8"]

# fp16 G-phase upper-triangle tile plan (baseline)
G_TILES = [
    (0, 0, 512), (0, 512, 512),
    (1, 128, 384), (1, 512, 512),
    (2, 256, 256), (2, 512, 512),
    (3, 384, 384), (3, 768, 256),
    (4, 512, 512),
    (5, 640, 384),
    (6, 768, 256),
    (7, 896, 128),
]

G_RECON = [
    (5, [4]),
    (6, [4, 5]),
    (7, [4, 5]),
    (7, [6]),
    (1, [0]),
    (2, [0, 1]),
    (3, [0, 1, 2]),
    (4, [0, 1, 2, 3]),
    (5, [0, 1, 2, 3]),
    (6, [0, 1, 2, 3]),
    (7, [0, 1, 2, 3]),
]


def _build(B, modes=None, static=False):
    import concourse.bacc as bacc
    import concourse.bass as bass
    import concourse.mybir as mybir
    from concourse.tile import TileContext
    from contextlib import nullcontext

    if modes is None:
        modes = MODES
    F32 = mybir.dt.float32
    F16 = mybir.dt.float16
    F8 = mybir.dt.float8e4
    U16 = mybir.dt.uint16
    COPY = mybir.ActivationFunctionType.Copy
    ADD = mybir.AluOpType.add
    SUB = mybir.AluOpType.subtract
    MULT = mybir.AluOpType.mult
    DR = mybir.MatmulPerfMode.DoubleRow

    PAIR = 2 if B % 2 == 0 else 1
    NPAIR = B // PAIR

    nc = bacc.Bacc("TRN2", target_bir_lowering=False, debug=False)
    w = nc.dram_tensor("w", [NPAIR, PAIR, NMAT, NMAT], F32, kind="ExternalInput")
    o = nc.dram_tensor("o", [NPAIR, PAIR, NMAT, NMAT], F32, kind="ExternalOutput")
    eye = nc.dram_tensor("eye", [P, P], F32, kind="ExternalInput")  # 1.5*I
    ideh = nc.dram_tensor("ideh", [P, P], F32, kind="ExternalInput")  # 0.5*I

    with TileContext(nc) as tc:
        with (
            tc.tile_pool(name="state", bufs=1) as st,
            tc.tile_pool(name="const", bufs=1) as cn,
            tc.tile_pool(name="tmp", bufs=3) as tp,
            tc.tile_pool(name="pg", bufs=4, space="PSUM") as pg,
            tc.tile_pool(name="pu", bufs=4, space="PSUM") as pu,
        ):
            eye32 = cn.tile([P, P], F32, tag="eye32")
            eye16 = cn.tile([P, P], F16, tag="eye16")  # 1.5 I
            eyeh32 = cn.tile([P, P], F32, tag="eyeh32")
            eyeh16 = cn.tile([P, P], F16, tag="eyeh16")  # 0.5 I
            nc.scalar.dma_start(eye32[:], eye.ap())
            nc.scalar.activation(eye16[:], eye32[:], COPY)
            nc.scalar.dma_start(eyeh32[:], ideh.ap())
            nc.scalar.activation(eyeh16[:], eyeh32[:], COPY, scale=2.0 * SM)
            id8 = cn.tile([P, P], F8, tag="id8")  # I (fp8, PE-transpose perm)
            nc.scalar.activation(id8[:], eyeh32[:], COPY, scale=2.0)
            eyeq16 = cn.tile([P, P], F16, tag="eyeq16")  # SM2 * I
            nc.scalar.activation(eyeq16[:], eyeh32[:], COPY, scale=2.0 * SM2)
            # zero fp8 tile: dummy start=True DoubleRow matmuls zero the PSUM
            # accumulation group (a data-carrying start=True DR matmul loses
            # its contribution on hardware)
            z8 = cn.tile([P, 1024], F8, tag="z8")
            nc.vector.memset(z8[:], 0)

            need16 = any(m == "16" for m in modes)

            def load(ib, s, V, nb2):
                # V = 32 * w  (fp16)
                for c in range(C):
                    tl = tp.tile([P, FB], F32, tag="tl", bufs=6)
                    nc.sync.dma_start(
                        tl[:],
                        w.ap()[
                            bass.ds(ib, 1),
                            s,
                            c * P : (c + 1) * P,
                            nb2 * FB : (nb2 + 1) * FB,
                        ],
                    )
                    dst = V[:, c * NMAT + nb2 * FB : c * NMAT + (nb2 + 1) * FB]
                    if c % 2 == 0:
                        nc.scalar.activation(dst, tl[:], COPY, scale=CS)
                    else:
                        nc.vector.tensor_scalar_mul(dst, tl[:], CS)

            def phase_T16(V, WT):
                WT3 = WT[:].rearrange("a (b c) -> a b c", b=C, c=NMAT)
                for i in range(C):
                    nc.sync.dma_start_transpose(
                        WT3[:, :, i * P : (i + 1) * P],
                        V[:, i * NMAT : (i + 1) * NMAT],
                    )

            def quant8(V, WT, W8, WT8, asq):
                # W8 = q8(asq*V), WT8 = q8(asq*WT)  (normal layouts)
                nc.vector.tensor_scalar_mul(W8[:, : 4 * NMAT], V[:, : 4 * NMAT], asq)
                nc.scalar.activation(
                    W8[:, 4 * NMAT :], V[:, 4 * NMAT :], COPY, scale=asq
                )
                nc.vector.tensor_scalar_mul(
                    WT8[:, : 4 * NMAT], WT[:, : 4 * NMAT], asq
                )
                nc.scalar.activation(
                    WT8[:, 4 * NMAT :], WT[:, 4 * NMAT :], COPY, scale=asq
                )

            def phase_G16(V, A, mode):
                # psum = V^T V = CS^2 G.
                # '16':    A = 1.5I - 0.5G   (diag +1.5I)
                # 'g16u8': A = SM*(I - G)    (diag +SM*I)
                dg = eye16 if mode == "16" else eyeh16
                for m, cs, wd in G_TILES:
                    g = pg.tile([P, FB], F32, tag="pg")
                    for k in range(C):
                        nc.tensor.matmul(
                            g[:, :wd],
                            V[:, k * NMAT + m * P : k * NMAT + (m + 1) * P],
                            V[:, k * NMAT + cs : k * NMAT + cs + wd],
                            start=(k == 0),
                            stop=(k == C - 1),
                        )
                    gsc = (-0.5 if mode == "16" else -SM) / (CS * CS)
                    nc.scalar.activation(
                        A[:, m * NMAT + cs : m * NMAT + cs + wd],
                        g[:, :wd],
                        COPY,
                        scale=gsc,
                    )
                    if cs <= m * P < cs + wd:
                        d = m * NMAT + m * P
                        nc.vector.tensor_tensor(
                            A[:, d : d + P], A[:, d : d + P], dg[:], ADD
                        )

            def phase_recon16(A):
                A3 = A[:].rearrange("a (b c) -> a b c", b=C, c=NMAT)
                for nb in range(C - 1):
                    nc.sync.dma_start_transpose(
                        A3[:, nb + 1 : C, nb * P : (nb + 1) * P],
                        A[:, nb * NMAT + (nb + 1) * P : (nb + 1) * NMAT],
                    )

            def phase_U16(ib, s, V, WT, A, nextmode, W8, WT8):
                # V' = V A (psum = V.A16, scale 1); drain into V in place
                for nb2 in (1, 0):
                    for i in range(C):
                        u = pu.tile([P, FB], F32, tag="pu")
                        for j in range(C):
                            nc.tensor.matmul(
                                u[:],
                                WT[:, j * NMAT + i * P : j * NMAT + (i + 1) * P],
                                A[:, j * NMAT + nb2 * FB : j * NMAT + (nb2 + 1) * FB],
                                start=(j == 0),
                                stop=(j == C - 1),
                            )
                        vdst = V[:, i * NMAT + nb2 * FB : i * NMAT + (nb2 + 1) * FB]
                        if i % 2 == 0:
                            nc.scalar.activation(vdst, u[:], COPY)
                        else:
                            nc.vector.tensor_copy(vdst, u[:])
                phase_T16(V, WT)
                if nextmode != "16":
                    quant8(V, WT, W8, WT8, _asq(nextmode))

            def phase_G8(W8, M8, mode="8", A=None, M8lo=None):
                # full G in fp8 DoubleRow ('hilo' path): A16 = SM2*(I-G) fp16,
                # then M8 = q8(A16), M8lo = q8(A16 - M8).
                sm = SM2 if mode == "hilo" else SM
                asx = AS2 if mode == "hilo" else AS
                g8sc = -sm / (asx * CS) ** 2
                W83 = W8[:].rearrange("p (c n) -> p c n", c=C)
                Z83 = z8[:].rearrange("p (c n) -> p c n", c=2)
                dst = A if mode == "hilo" else M8
                dg = eyeq16 if mode == "hilo" else eyeh16
                for m in range(C):
                    for h in range(2):
                        g = pg.tile([P, FB], F32, tag="pg")
                        for ng in range(2):
                            nc.tensor.matmul(
                                g[:, ng * 256 : (ng + 1) * 256],
                                Z83[:, :, 0:P],
                                Z83[:, :, 0:256],
                                start=True,
                                stop=False,
                                perf_mode=DR,
                            )
                        for j2 in range(4):
                            for ng in range(2):
                                nc.tensor.matmul(
                                    g[:, ng * 256 : (ng + 1) * 256],
                                    W83[:, 2 * j2 : 2 * j2 + 2, m * P : (m + 1) * P],
                                    W83[
                                        :,
                                        2 * j2 : 2 * j2 + 2,
                                        h * FB + ng * 256 : h * FB + (ng + 1) * 256,
                                    ],
                                    start=False,
                                    stop=(j2 == 3),
                                    perf_mode=DR,
                                )
                        base = m * NMAT + h * FB
                        if mode == "hilo":
                            if m % 4 == 3:
                                nc.vector.tensor_scalar_mul(
                                    dst[:, base : base + FB], g[:], g8sc
                                )
                            else:
                                nc.scalar.activation(
                                    dst[:, base : base + FB], g[:], COPY, scale=g8sc
                                )
                            if m // 4 == h:
                                d = base + (m % 4) * P
                                nc.gpsimd.tensor_tensor(
                                    dst[:, d : d + P], dst[:, d : d + P], dg[:], ADD
                                )
                        elif m // 4 == h:
                            d0 = (m % 4) * P
                            if d0 > 0:
                                nc.scalar.activation(
                                    M8[:, base : base + d0],
                                    g[:, :d0],
                                    COPY,
                                    scale=g8sc,
                                )
                            if d0 + P < FB:
                                nc.scalar.activation(
                                    M8[:, base + d0 + P : base + FB],
                                    g[:, d0 + P : FB],
                                    COPY,
                                    scale=g8sc,
                                )
                            td = tp.tile([P, P], F16, tag="td", bufs=4)
                            nc.scalar.activation(
                                td[:], g[:, d0 : d0 + P], COPY, scale=g8sc
                            )
                            nc.gpsimd.tensor_tensor(
                                M8[:, base + d0 : base + d0 + P], td[:], eyeh16[:], ADD
                            )
                        elif m in (3, 6):
                            nc.vector.tensor_scalar_mul(
                                M8[:, base : base + FB], g[:], g8sc
                            )
                        else:
                            nc.scalar.activation(
                                M8[:, base : base + FB], g[:], COPY, scale=g8sc
                            )
                if mode == "hilo":
                    nc.gpsimd.tensor_copy(M8[:, : 4 * NMAT], A[:, : 4 * NMAT])
                    nc.scalar.activation(M8[:, 4 * NMAT :], A[:, 4 * NMAT :], COPY)
                    nc.vector.tensor_tensor(
                        M8lo[:, : 4 * NMAT], A[:, : 4 * NMAT], M8[:, : 4 * NMAT], SUB
                    )
                    nc.gpsimd.tensor_tensor(
                        M8lo[:, 4 * NMAT :], A[:, 4 * NMAT :], M8[:, 4 * NMAT :], SUB
                    )

            def phase_G8tri16(W8, A):
                # triangle G from fp8 DoubleRow, drained to fp16
                # A = 1.5I - 0.5G for a fp16 U phase; lower blocks via the
                # fp16 XBAR recon (phase_recon16)
                gsc = -0.5 / (AS * CS) ** 2
                W83 = W8[:].rearrange("p (c n) -> p c n", c=C)
                Z83 = z8[:].rearrange("p (c n) -> p c n", c=2)
                for m, cs, wd in G_TILES:
                    g = pg.tile([P, FB], F32, tag="pg")
                    groups = [(0, min(256, wd))]
                    if wd > 256:
                        groups.append((256, wd - 256))
                    for go, gw in groups:
                        nc.tensor.matmul(
                            g[:, go : go + gw],
                            Z83[:, :, 0:P],
                            Z83[:, :, 0:gw],
                            start=True,
                            stop=False,
                            perf_mode=DR,
                        )
                    for j2 in range(4):
                        for go, gw in groups:
                            nc.tensor.matmul(
                                g[:, go : go + gw],
                                W83[:, 2 * j2 : 2 * j2 + 2, m * P : (m + 1) * P],
                                W83[:, 2 * j2 : 2 * j2 + 2, cs + go : cs + go + gw],
                                start=False,
                                stop=(j2 == 3),
                                perf_mode=DR,
                            )
                    nc.scalar.activation(
                        A[:, m * NMAT + cs : m * NMAT + cs + wd],
                        g[:, :wd],
                        COPY,
                        scale=gsc,
                    )
                    if cs <= m * P < cs + wd:
                        d = m * NMAT + m * P
                        nc.vector.tensor_tensor(
                            A[:, d : d + P], A[:, d : d + P], eye16[:], ADD
                        )

            def phase_G8tri(W8, M8):
                # upper-triangle G in fp8 DoubleRow; strict-lower 128-blocks
                # reconstructed by PE transposes of their upper mirrors
                # (fp8 transpose lands in u16 lanes: psum stride-2 drain),
                # interleaved with the G-tile stream row by row.
                g8sc = -SM / (AS * CS) ** 2
                W83 = W8[:].rearrange("p (c n) -> p c n", c=C)
                Z83 = z8[:].rearrange("p (c n) -> p c n", c=2)

                def mirror(mb):
                    t = pg.tile([P, FB], F32, tag="pg")
                    t8 = t[:].bitcast(F8)
                    for nb in range(mb):
                        nc.tensor.matmul(
                            t8[:, nb * 256 : nb * 256 + 256].rearrange(
                                "p (n two) -> p n two", two=2
                            )[:, :, 0],
                            M8[:, nb * NMAT + mb * P : nb * NMAT + (mb + 1) * P],
                            id8[:],
                            is_transpose=True,
                        )
                    srcap = t8[:, 0 : mb * 256].rearrange(
                        "p (n two) -> p n two", two=2
                    )[:, :, 0]
                    dst = M8[:, mb * NMAT : mb * NMAT + mb * P]
                    if mb % 2 == 0:
                        nc.vector.tensor_copy(dst, srcap)
                    else:
                        nc.scalar.activation(dst, srcap, COPY)

                for m, cs, wd in G_TILES:
                    g = pg.tile([P, FB], F32, tag="pg")
                    groups = [(0, min(256, wd))]
                    if wd > 256:
                        groups.append((256, wd - 256))
                    for go, gw in groups:
                        nc.tensor.matmul(
                            g[:, go : go + gw],
                            Z83[:, :, 0:P],
                            Z83[:, :, 0:gw],
                            start=True,
                            stop=False,
                            perf_mode=DR,
                        )
                    for j2 in range(4):
                        for go, gw in groups:
                            nc.tensor.matmul(
                                g[:, go : go + gw],
                                W83[:, 2 * j2 : 2 * j2 + 2, m * P : (m + 1) * P],
                                W83[:, 2 * j2 : 2 * j2 + 2, cs + go : cs + go + gw],
                                start=False,
                                stop=(j2 == 3),
                                perf_mode=DR,
                            )
                    base = m * NMAT + cs
                    if cs <= m * P < cs + wd:
                        d0 = m * P - cs
                        if d0 > 0:
                            nc.scalar.activation(
                                M8[:, base : base + d0], g[:, :d0], COPY, scale=g8sc
                            )
                        if d0 + P < wd:
                            nc.scalar.activation(
                                M8[:, base + d0 + P : base + wd],
                                g[:, d0 + P : wd],
                                COPY,
                                scale=g8sc,
                            )
                        td = tp.tile([P, P], F16, tag="td", bufs=4)
                        nc.scalar.activation(
                            td[:], g[:, d0 : d0 + P], COPY, scale=g8sc
                        )
                        nc.gpsimd.tensor_tensor(
                            M8[:, base + d0 : base + d0 + P], td[:], eyeh16[:], ADD
                        )
                    elif m in (2, 5):
                        nc.vector.tensor_scalar_mul(
                            M8[:, base : base + wd], g[:, :wd], g8sc
                        )
                    else:
                        nc.scalar.activation(
                            M8[:, base : base + wd], g[:, :wd], COPY, scale=g8sc
                        )
                for mb in range(1, C):
                    mirror(mb)

            def phase_U8(ib, s, V, WT, WT8, M8, W8, last, asq=AS, M8lo=None):
                # psum = W8 @ M8 = 0.5 V (I-G) exactly; V += psum (DVE).
                # lhsT = WT8 chunk-pairs, rhs = M8 chunk-pairs (DoubleRow).
                # Per-chunk: as soon as V chunk i is updated, transpose it
                # into WT16 (XBAR DMA) and quantize W8 chunk i, so the next
                # iteration's operands are ready with minimal tail latency.
                WT83 = WT8[:].rearrange("p (c n) -> p c n", c=C)
                M83 = M8[:].rearrange("p (c n) -> p c n", c=C)
                ML3 = M8lo[:].rearrange("p (c n) -> p c n", c=C) if M8lo is not None else None
                Z83 = z8[:].rearrange("p (c n) -> p c n", c=2)
                WT3 = WT[:].rearrange("a (b c) -> a b c", b=C, c=NMAT)
                for i in range(C):
                    for h in range(2):
                        u = pu.tile([P, FB], F32, tag="pu")
                        for ng in range(2):
                            nc.tensor.matmul(
                                u[:, ng * 256 : (ng + 1) * 256],
                                Z83[:, :, 0:P],
                                Z83[:, :, 0:256],
                                start=True,
                                stop=False,
                                perf_mode=DR,
                            )
                        srcs = [M83] if ML3 is None else [M83, ML3]
                        for si, MS in enumerate(srcs):
                            for dd in range(4):
                                for ng in range(2):
                                    ns = h * FB + ng * 256
                                    nc.tensor.matmul(
                                        u[:, ng * 256 : (ng + 1) * 256],
                                        WT83[
                                            :, 2 * dd : 2 * dd + 2, i * P : (i + 1) * P
                                        ],
                                        MS[:, 2 * dd : 2 * dd + 2, ns : ns + 256],
                                        start=False,
                                        stop=(si == len(srcs) - 1 and dd == 3),
                                        perf_mode=DR,
                                    )
                        vdst = V[:, i * NMAT + h * FB : i * NMAT + (h + 1) * FB]
                        nc.vector.tensor_tensor(vdst, vdst, u[:], ADD)
                        if last:
                            t32 = tp.tile([P, FB], F32, tag="t32", bufs=6)
                            nc.scalar.activation(t32[:], vdst, COPY, scale=1.0 / CS)
                            oap = o.ap()[
                                bass.ds(ib, 1),
                                s,
                                i * P : (i + 1) * P,
                                h * FB : (h + 1) * FB,
                            ]
                            if i % 2 == 0:
                                nc.sync.dma_start(oap, t32[:])
                            else:
                                nc.scalar.dma_start(oap, t32[:])
                    if not last:
                        nc.sync.dma_start_transpose(
                            WT3[:, :, i * P : (i + 1) * P],
                            V[:, i * NMAT : (i + 1) * NMAT],
                        )
                        w8dst = W8[:, i * NMAT : (i + 1) * NMAT]
                        w8src = V[:, i * NMAT : (i + 1) * NMAT]
                        if i in (1, 5):
                            nc.scalar.activation(w8dst, w8src, COPY, scale=asq)
                        elif i in (3, 7):
                            nc.vector.tensor_scalar_mul(w8dst, w8src, asq)
                        else:
                            nc.gpsimd.tensor_scalar_mul(w8dst, w8src, asq)
                if not last:
                    nc.scalar.activation(
                        WT8[:, : 2 * NMAT], WT[:, : 2 * NMAT], COPY, scale=asq
                    )
                    nc.scalar.activation(
                        WT8[:, 2 * NMAT : 4 * NMAT],
                        WT[:, 2 * NMAT : 4 * NMAT],
                        COPY,
                        scale=asq,
                    )
                    nc.vector.tensor_scalar_mul(
                        WT8[:, 4 * NMAT : 6 * NMAT], WT[:, 4 * NMAT : 6 * NMAT], asq
                    )
                    nc.vector.tensor_scalar_mul(
                        WT8[:, 6 * NMAT :], WT[:, 6 * NMAT :], asq
                    )

            loop_cm = nullcontext(0) if static else tc.For_i(0, NPAIR)
            with loop_cm as ib:
              for _sib in range(NPAIR if static else 1):
                if static:
                    ib = _sib
                Vs, WTs, As, W8s, WT8s, M8s = [], [], [], [], [], []
                M8los = []
                for s in range(PAIR):
                    Vs.append(st.tile([P, C * NMAT], F16, tag=f"V{s}", name=f"V{s}"))
                    W8s.append(st.tile([P, C * NMAT], F8, tag=f"W8{s}", name=f"W8{s}"))
                    WT8s.append(
                        st.tile([P, C * NMAT], F8, tag=f"WT8{s}", name=f"WT8{s}")
                    )
                    M8s.append(st.tile([P, C * NMAT], F8, tag=f"M8{s}", name=f"M8{s}"))
                    M8los.append(
                        st.tile([P, C * NMAT], F8, tag=f"M8lo{s}", name=f"M8lo{s}")
                    )
                    WTs.append(st.tile([P, C * NMAT], F16, tag=f"WT{s}", name=f"WT{s}"))
                    As.append(st.tile([P, C * NMAT], F16, tag=f"A{s}", name=f"A{s}"))
                for s in range(PAIR):
                    load(ib, s, Vs[s], 1)
                for s in range(PAIR):
                    load(ib, s, Vs[s], 0)
                for s in range(PAIR):
                    phase_T16(Vs[s], WTs[s])
                    if modes[0] != "16":
                        quant8(Vs[s], WTs[s], W8s[s], WT8s[s], _asq(modes[0]))

                it = 0
                while it < len(modes):
                    mode = modes[it]
                    last = it == len(modes) - 1
                    nextmode = modes[it + 1] if not last else None
                    if mode in ("8", "hilo") and PAIR == 2:
                        run = 1
                        while it + run < len(modes) and modes[it + run] in (
                            "8",
                            "hilo",
                        ):
                            run += 1
                        # software-pipelined pair emission: G(s1,t) U(s0,t)
                        # U(s1,t) G(s0,t+1): every cross-engine dependency
                        # (drains, quantize, transposes) is covered by a
                        # full phase of the partner matrix on the PE queue.
                        def g8(s, t):
                            if modes[t] == "8":
                                phase_G8tri(W8s[s], M8s[s])
                            else:
                                phase_G8(
                                    W8s[s],
                                    M8s[s],
                                    modes[t],
                                    As[s],
                                    M8los[s],
                                )

                        def u8(s, t):
                            lastj = t == len(modes) - 1
                            nxt = modes[t + 1] if not lastj else "8"
                            phase_U8(
                                ib,
                                s,
                                Vs[s],
                                WTs[s],
                                WT8s[s],
                                M8s[s],
                                W8s[s],
                                lastj,
                                _asq(nxt),
                                M8los[s] if modes[t] == "hilo" else None,
                            )

                        g8(0, it)
                        for j in range(run):
                            t = it + j
                            g8(1, t)
                            u8(0, t)
                            if j + 1 < run:
                                g8(0, t + 1)
                            u8(1, t)
                        it += run
                        continue
                    if mode in ("16", "g8u16"):
                        for s in range(PAIR):
                            if mode == "16":
                                phase_G16(Vs[s], As[s], mode)
                            else:
                                phase_G8tri16(W8s[s], As[s])
                        for s in range(PAIR):
                            phase_recon16(As[s])
                        for s in range(PAIR):
                            phase_U16(
                                ib, s, Vs[s], WTs[s], As[s], nextmode, W8s[s], WT8s[s]
                            )
                        assert not last, "last iter must be fp8-typed or add store"
                    elif mode == "g16u8":
                        # per-matrix chains interleaved: s0's recon/cast run
                        # during s1's G16 so U8(s0) starts as G16(s1) ends
                        for s in range(PAIR):
                            phase_G16(Vs[s], As[s], mode)
                            phase_recon16(As[s])
                            # M8 = cast(A16) = q8(SM*(I-G)) full normal
                            nc.gpsimd.tensor_copy(
                                M8s[s][:, : 4 * NMAT], As[s][:, : 4 * NMAT]
                            )
                            nc.scalar.activation(
                                M8s[s][:, 4 * NMAT :], As[s][:, 4 * NMAT :], COPY
                            )
                        for s in range(PAIR):
                            phase_U8(
                                ib, s, Vs[s], WTs[s], WT8s[s], M8s[s], W8s[s], last
                            )
                    elif mode in ("8", "hilo"):
                        for s in range(PAIR):
                            if mode == "8":
                                phase_G8tri(W8s[s], M8s[s])
                            else:
                                phase_G8(W8s[s], M8s[s], mode, As[s], M8los[s])
                        for s in range(PAIR):
                            phase_U8(
                                ib,
                                s,
                                Vs[s],
                                WTs[s],
                                WT8s[s],
                                M8s[s],
                                W8s[s],
                                last,
                                _asq(nextmode) if nextmode else AS,
                                M8los[s] if mode == "hilo" else None,
                            )
                    else:
                        raise ValueError(mode)
                    it += 1
    nc.compile()
    return nc


def _get_nc(B):
    key = (B, tuple(MODES))
    if key not in _NC_CACHE:
        _NC_CACHE[key] = _build(B)
    return _NC_CACHE[key]


def kernel(w) -> np.ndarray:
    from concourse.bass_utils import run_bass_kernel_spmd

    w = np.ascontiguousarray(np.asarray(w, dtype=np.float32))
    assert w.shape == (64, NMAT, NMAT), w.shape
    B = 8
    nc = _get_nc(B)
    eye15 = (1.5 * np.eye(P)).astype(np.float32)
    eyeh = (0.5 * np.eye(P)).astype(np.float32)
    in_maps = [
        {
            "w": np.ascontiguousarray(w[c * B : (c + 1) * B]).reshape(
                B // 2, 2, NMAT, NMAT
            ),
            "eye": eye15,
            "ideh": eyeh,
        }
        for c in range(8)
    ]
    res = run_bass_kernel_spmd(nc, in_maps, core_ids=list(range(8)))
    return np.concatenate(
        [res.results[c]["o"].reshape(B, NMAT, NMAT) for c in range(8)], axis=0
    )
